# revision 37
# baseline (speedup 1.0000x reference)
"""TRN2 Bass kernel for nn_CosClassifier: sim = 10*scalar * cos_sim(inputs, proto).

Data-parallel over 8 NeuronCores: each core computes a (2048, 4096) slab of
the (16384, 4096) similarity matrix as one fp16 GEMM. The device does all the
O(B*C*D) work -- the 17 GFLOP matmul, the 8.4M-element scaled PSUM drains and
every byte of output DMA; the host does O(input-size) preparation (sharding,
inverse norms, operand layout/dtype prep, fp16->f32 upcast of the result).

v6 shape, driven by trace analysis of on-device-prep versions (which plateaued
at ~97us):
  1. Output is fp16 in DRAM (16MB/core, the roofline stream) and upcast on the
     host; result quantization adds ~5e-4 relative error, inside the gate.
  2. Operands are uploaded GEMM-ready: x and proto pre-transposed to the
     [contract-dim partition, tile, k, free] stationary/moving layouts the PE
     wants, fp16, proto rows pre-scaled by scalar/||p||. This removes the PE
     transposes (8.4us), operand casts and proto scaling that previously kept
     ACT/DVE/PE mutually blocking, and halves input wire traffic (3MB/core).
  3. 10/||x_b|| is applied on-device in the PSUM->SBUF drains (per-partition
     activation scale, same cost as a plain copy). b-tiles are drained in
     PAIRS into one [128, 2048] staging tile -- ACT takes one half, DVE the
     other (different PSUM banks, legal in parallel) -- and ONE 512KB DMA
     ships the pair: half the ring issues/semaphores of per-tile DMAs.
  4. A burst of matmuls on a memset scratch tile warms the PE HAM clock gate
     (1.2 -> 2.4 GHz needs ~3.4us of sustained activity) while the first
     operand chunks stream in, so the real GEMM starts warm at ~11us.
"""
import sys

sys.path.insert(0, "/opt/trn_rl_repo")

import numpy as np

B, C, D = 16384, 4096, 256
NCORES = 8
BS = B // NCORES          # 2048 rows per core
NB = BS // 128            # 16 b-tiles per core
NCT = C // 128            # 32 c-tiles (proto rows)
NK = D // 128             # 2 k-tiles
NPH = 4                   # output column phases (1024 wide each)

_compiled = None


def _build():
    import concourse.bacc as bacc
    import concourse.mybir as mybir
    import concourse.tile as tile

    f32 = mybir.dt.float32
    f16 = mybir.dt.float16
    Act = mybir.ActivationFunctionType

    nc = bacc.Bacc("TRN2", target_bir_lowering=False, debug=False,
                   num_devices=NCORES)

    # xt[p, i, k, f] = x[i*128+f, k*128+p] : stationary layout, fp16
    xt_d = nc.dram_tensor("xt", [128, NB * NK * 128], f16,
                          kind="ExternalInput").ap()
    # pt[p, j, k, f] = (scalar/||p_c||)*proto[j*128+f, k*128+p] : moving
    pt_d = nc.dram_tensor("pt", [128, NCT * NK * 128], f16,
                          kind="ExternalInput").ap()
    xi_d = nc.dram_tensor("xinv", [128, NB], f32, kind="ExternalInput").ap()
    out_d = nc.dram_tensor("out", [BS, C], f16, kind="ExternalOutput").ap()

    with tile.TileContext(nc) as tc:
        with tc.tile_pool(name="sbuf", bufs=1) as pool, \
             tc.tile_pool(name="outp", bufs=8) as outp, \
             tc.tile_pool(name="psum_w", bufs=1, space="PSUM") as psum_w, \
             tc.tile_pool(name="psum_m", bufs=3, space="PSUM") as psum_m:

            xinv = pool.tile([128, NB], f32, tag="xinv")
            nc.scalar.dma_start(xinv[:], xi_d[:, :])

            # PE HAM warm-up on a memset scratch tile: no DMA dependency, so
            # the clock gate lifts to 2.4GHz while operands stream in
            wscr = pool.tile([128, 128], f16, tag="wscr")
            nc.gpsimd.memset(wscr[:], 0.0)
            wp = psum_w.tile([128, 512], f32, tag="wp")
            for _ in range(20):
                nc.tensor.matmul(wp[:, 0:128], wscr[:], wscr[:],
                                 start=True, stop=True)

            # ONE TILE PER DMA CHUNK: dependency tracking is tile-granular,
            # so a shared tile would make the first matmul wait for the LAST
            # chunk. All loads ride the sync ring in first-use order (a
            # second ring just contends for the same HBM wire); the first
            # matmul needs only xt chunk 0 (b-tiles 0-3) and pt chunk 0a
            # (n-block 0), 256KB each, so the GEMM starts at ~9.5us.
            # xt chunk tile-starts (in b-tiles): the first chunk is a single
            # 64KB tile so the very first matmul's stationary lands earliest
            XS = [0, 1, 4, 8, 12, 16]
            xtc = [pool.tile([128, (XS[c + 1] - XS[c]) * NK * 128], f16,
                             tag=f"xt{c}", name=f"xt{c}")
                   for c in range(len(XS) - 1)]
            ptc = [pool.tile([128, 1024 if h < 2 else 2048], f16,
                             tag=f"pt{h}", name=f"pt{h}")
                   for h in range(NPH + 1)]
            nc.sync.dma_start(xtc[0][:], xt_d[:, :256])
            nc.sync.dma_start(ptc[0][:], pt_d[:, :1024])
            nc.sync.dma_start(ptc[1][:], pt_d[:, 1024:2048])
            for c in range(1, len(XS) - 1):
                nc.sync.dma_start(
                    xtc[c][:], xt_d[:, XS[c] * 256:XS[c + 1] * 256])
            for h in range(1, NPH):
                nc.sync.dma_start(ptc[h + 1][:],
                                  pt_d[:, h * 2048:(h + 1) * 2048])
            chunk_of = []
            local_of = []
            for c in range(len(XS) - 1):
                for i in range(XS[c], XS[c + 1]):
                    chunk_of.append(c)
                    local_of.append(i - XS[c])

            xtc_r = [t[:].rearrange("p (i two d) -> p i two d", two=NK, d=128)
                     for t in xtc]
            ptc_r = [t[:].rearrange("p (j two d) -> p j two d", two=NK, d=128)
                     for t in ptc]

            def moving(h, nn_):
                # phase 0 n-blocks live in two half tiles; later phases in
                # one tile per phase
                if h == 0:
                    return ptc_r[nn_][:, 0:4, :, :]
                return ptc_r[h + 1][:, 4 * nn_:4 * nn_ + 4, :, :]

            # ---- main matmul + scaled fp16 drain ----
            # phase h covers n-blocks {2h, 2h+1}; per b-tile i: 4 fp16 MMs
            # (k-outer) into a 2-bank PSUM tile, one 1024-wide drain applying
            # 10/||x_b|| and casting to fp16. b-tiles are paired: ACT drains
            # one half, DVE the other (parallel, different banks), one 512KB
            # DMA per pair.
            def mm(h, i, oq2):
                ps = psum_m.tile([128, 1024], f32, tag="mm")
                for nn_ in range(2):
                    mv = moving(h, nn_)
                    for k in range(NK):
                        nc.tensor.matmul(
                            ps[:, nn_ * 512:(nn_ + 1) * 512],
                            xtc_r[chunk_of[i]][:, local_of[i], k, :],
                            mv[:, :, k, :],
                            start=(k == 0), stop=(k == NK - 1))
                inv = xinv[:, i:i + 1]
                half = oq2[:, (i % 2) * 1024:(i % 2 + 1) * 1024]
                if i % 2 == 0:
                    nc.scalar.activation(half, ps[:], Act.Copy, scale=inv)
                else:
                    nc.vector.tensor_scalar_mul(half, ps[:], inv)

            for h in range(NPH):
                for pr in range(NB // 2):
                    oq2 = outp.tile([128, 2048], f16, tag="oq")
                    for half in range(2):
                        mm(h, 2 * pr + half, oq2)
                    nc.sync.dma_start(
                        out_d[2 * pr * 128:(2 * pr + 2) * 128,
                              h * 1024:(h + 1) * 1024].rearrange(
                                  "(n p) c -> p n c", p=128),
                        oq2[:].rearrange("p (n c) -> p n c", c=1024))

    nc.compile()
    return nc


def _get_compiled():
    global _compiled
    if _compiled is None:
        _compiled = _build()
    return _compiled


def kernel(inputs, proto, scalar, _trace=False, **_tr_kw):
    from concourse.bass_utils import run_bass_kernel_spmd

    nc = _get_compiled()
    inputs = np.ascontiguousarray(inputs, dtype=np.float32)
    proto = np.ascontiguousarray(proto, dtype=np.float32)
    sc = float(np.asarray(scalar).reshape(-1)[0])

    # O(input-size) prep: inverse norms, proto pre-scale, and the PE
    # stationary/moving fp16 layouts ([p, tile, k, f], contract dim on the
    # partition axis)
    pnorm = np.linalg.norm(proto.astype(np.float64), axis=1)
    p_scaled = (proto * (sc / pnorm).astype(np.float32)[:, None])
    pt = np.ascontiguousarray(
        p_scaled.reshape(NCT, 128, NK, 128).transpose(3, 0, 2, 1)
    ).astype(np.float16).reshape(128, NCT * NK * 128)
    xnorm = np.linalg.norm(inputs.astype(np.float64), axis=1)
    xinv_full = (10.0 / xnorm).astype(np.float32)

    in_maps = []
    for c in range(NCORES):
        xs = inputs[c * BS:(c + 1) * BS]
        xt = np.ascontiguousarray(
            xs.reshape(NB, 128, NK, 128).transpose(3, 0, 2, 1)
        ).astype(np.float16).reshape(128, NB * NK * 128)
        xinv = xinv_full[c * BS:(c + 1) * BS].reshape(NB, 128).T.copy()
        in_maps.append({"xt": xt, "pt": pt, "xinv": xinv})
    res = run_bass_kernel_spmd(nc, in_maps, core_ids=list(range(NCORES)),
                               trace=_trace, **_tr_kw)
    out = np.concatenate([res.results[c]["out"] for c in range(NCORES)],
                         axis=0).astype(np.float32)
    if _trace:
        kernel.last_results = res
    return out


# revision 38
# speedup vs baseline: 1.2277x; 1.2277x over previous
"""TRN2 Bass kernel for nn_CosClassifier: sim = 10*scalar * cos_sim(inputs, proto).

Data-parallel over 8 NeuronCores: each core computes a (2048, 4096) slab of
the (16384, 4096) similarity matrix as one fp16 GEMM. The device does all the
O(B*C*D) work -- the 17 GFLOP matmul, the 8.4M-element scaled PSUM drains and
every byte of output DMA; the host does O(input-size) preparation (sharding,
inverse norms, operand layout/dtype prep, fp16->f32 upcast of the result).

v6 shape, driven by trace analysis of on-device-prep versions (which plateaued
at ~97us):
  1. Output is fp16 in DRAM (16MB/core, the roofline stream) and upcast on the
     host; result quantization adds ~5e-4 relative error, inside the gate.
  2. Operands are uploaded GEMM-ready: x and proto pre-transposed to the
     [contract-dim partition, tile, k, free] stationary/moving layouts the PE
     wants, fp16, proto rows pre-scaled by scalar/||p||. This removes the PE
     transposes (8.4us), operand casts and proto scaling that previously kept
     ACT/DVE/PE mutually blocking, and halves input wire traffic (3MB/core).
  3. 10/||x_b|| is applied on-device in the PSUM->SBUF drains (per-partition
     activation scale, same cost as a plain copy). b-tiles are drained in
     PAIRS into one [128, 2048] staging tile -- ACT takes one half, DVE the
     other (different PSUM banks, legal in parallel) -- and ONE 512KB DMA
     ships the pair: half the ring issues/semaphores of per-tile DMAs.
  4. A burst of matmuls on a memset scratch tile warms the PE HAM clock gate
     (1.2 -> 2.4 GHz needs ~3.4us of sustained activity) while the first
     operand chunks stream in, so the real GEMM starts warm at ~11us.
"""
import sys

sys.path.insert(0, "/opt/trn_rl_repo")

import numpy as np

B, C, D = 16384, 4096, 256
NCORES = 8
BS = B // NCORES          # 2048 rows per core
NB = BS // 128            # 16 b-tiles per core
NCT = C // 128            # 32 c-tiles (proto rows)
NK = D // 128             # 2 k-tiles
NPH = 4                   # output column phases (1024 wide each)

_compiled = None


def _build():
    import concourse.bacc as bacc
    import concourse.mybir as mybir
    import concourse.tile as tile

    f32 = mybir.dt.float32
    f16 = mybir.dt.float16
    Act = mybir.ActivationFunctionType

    nc = bacc.Bacc("TRN2", target_bir_lowering=False, debug=False,
                   num_devices=NCORES)

    # xt[p, i, k, f] = x[i*128+f, k*128+p] : stationary layout, fp16
    xt_d = nc.dram_tensor("xt", [128, NB * NK * 128], f16,
                          kind="ExternalInput").ap()
    # pt[p, j, k, f] = (scalar/||p_c||)*proto[j*128+f, k*128+p] : moving
    pt_d = nc.dram_tensor("pt", [128, NCT * NK * 128], f16,
                          kind="ExternalInput").ap()
    xi_d = nc.dram_tensor("xinv", [128, NB], f32, kind="ExternalInput").ap()
    out_d = nc.dram_tensor("out", [BS, C], f16, kind="ExternalOutput").ap()

    with tile.TileContext(nc) as tc:
        with tc.tile_pool(name="sbuf", bufs=1) as pool, \
             tc.tile_pool(name="outp", bufs=8) as outp, \
             tc.tile_pool(name="psum_w", bufs=1, space="PSUM") as psum_w, \
             tc.tile_pool(name="psum_m", bufs=3, space="PSUM") as psum_m:

            xinv = pool.tile([128, NB], f32, tag="xinv")
            nc.scalar.dma_start(xinv[:], xi_d[:, :])

            # PE HAM warm-up on a memset scratch tile: no DMA dependency, so
            # the clock gate lifts to 2.4GHz while operands stream in
            wscr = pool.tile([128, 128], f16, tag="wscr")
            nc.gpsimd.memset(wscr[:], 0.0)
            wp = psum_w.tile([128, 512], f32, tag="wp")
            for _ in range(32):
                nc.tensor.matmul(wp[:, 0:128], wscr[:], wscr[:],
                                 start=True, stop=True)

            # ONE TILE PER DMA CHUNK: dependency tracking is tile-granular,
            # so a shared tile would make the first matmul wait for the LAST
            # chunk. All loads ride the sync ring in first-use order (a
            # second ring just contends for the same HBM wire); the first
            # matmul needs only xt chunk 0 (b-tiles 0-3) and pt chunk 0a
            # (n-block 0), 256KB each, so the GEMM starts at ~9.5us.
            # xt chunk tile-starts (in b-tiles): 4 tiles per chunk; smaller
            # first chunks lose more to the ~0.7us flat ring-issue cost per
            # DMA (which delays the output stream) than they gain in arrival
            XS = [0, 4, 8, 12, 16]
            xtc = [pool.tile([128, (XS[c + 1] - XS[c]) * NK * 128], f16,
                             tag=f"xt{c}", name=f"xt{c}")
                   for c in range(len(XS) - 1)]
            ptc = [pool.tile([128, 1024 if h < 2 else 2048], f16,
                             tag=f"pt{h}", name=f"pt{h}")
                   for h in range(NPH + 1)]
            nc.sync.dma_start(xtc[0][:], xt_d[:, :XS[1] * 256])
            nc.sync.dma_start(ptc[0][:], pt_d[:, :1024])
            nc.sync.dma_start(ptc[1][:], pt_d[:, 1024:2048])
            for c in range(1, len(XS) - 1):
                nc.sync.dma_start(
                    xtc[c][:], xt_d[:, XS[c] * 256:XS[c + 1] * 256])
            for h in range(1, NPH):
                nc.sync.dma_start(ptc[h + 1][:],
                                  pt_d[:, h * 2048:(h + 1) * 2048])
            chunk_of = []
            local_of = []
            for c in range(len(XS) - 1):
                for i in range(XS[c], XS[c + 1]):
                    chunk_of.append(c)
                    local_of.append(i - XS[c])

            xtc_r = [t[:].rearrange("p (i two d) -> p i two d", two=NK, d=128)
                     for t in xtc]
            ptc_r = [t[:].rearrange("p (j two d) -> p j two d", two=NK, d=128)
                     for t in ptc]

            def moving(h, nn_):
                # phase 0 n-blocks live in two half tiles; later phases in
                # one tile per phase
                if h == 0:
                    return ptc_r[nn_][:, 0:4, :, :]
                return ptc_r[h + 1][:, 4 * nn_:4 * nn_ + 4, :, :]

            # ---- main matmul + scaled fp16 drain ----
            # phase h covers n-blocks {2h, 2h+1}; per b-tile i: 4 fp16 MMs
            # (k-outer) into a 2-bank PSUM tile, one 1024-wide drain applying
            # 10/||x_b|| and casting to fp16. b-tiles are paired: ACT drains
            # one half, DVE the other (parallel, different banks), one 512KB
            # DMA per pair.
            def mm(h, i, oq2):
                ps = psum_m.tile([128, 1024], f32, tag="mm")
                for nn_ in range(2):
                    mv = moving(h, nn_)
                    for k in range(NK):
                        nc.tensor.matmul(
                            ps[:, nn_ * 512:(nn_ + 1) * 512],
                            xtc_r[chunk_of[i]][:, local_of[i], k, :],
                            mv[:, :, k, :],
                            start=(k == 0), stop=(k == NK - 1))
                inv = xinv[:, i:i + 1]
                half = oq2[:, (i % 2) * 1024:(i % 2 + 1) * 1024]
                if i % 2 == 0:
                    nc.scalar.activation(half, ps[:], Act.Copy, scale=inv)
                else:
                    nc.vector.tensor_scalar_mul(half, ps[:], inv)

            for h in range(NPH):
                for pr in range(NB // 2):
                    oq2 = outp.tile([128, 2048], f16, tag="oq")
                    for half in range(2):
                        mm(h, 2 * pr + half, oq2)
                    nc.sync.dma_start(
                        out_d[2 * pr * 128:(2 * pr + 2) * 128,
                              h * 1024:(h + 1) * 1024].rearrange(
                                  "(n p) c -> p n c", p=128),
                        oq2[:].rearrange("p (n c) -> p n c", c=1024))

    nc.compile()
    return nc


def _get_compiled():
    global _compiled
    if _compiled is None:
        _compiled = _build()
    return _compiled


def kernel(inputs, proto, scalar, _trace=False, **_tr_kw):
    from concourse.bass_utils import run_bass_kernel_spmd

    nc = _get_compiled()
    inputs = np.ascontiguousarray(inputs, dtype=np.float32)
    proto = np.ascontiguousarray(proto, dtype=np.float32)
    sc = float(np.asarray(scalar).reshape(-1)[0])

    # O(input-size) prep: inverse norms, proto pre-scale, and the PE
    # stationary/moving fp16 layouts ([p, tile, k, f], contract dim on the
    # partition axis)
    pnorm = np.linalg.norm(proto.astype(np.float64), axis=1)
    p_scaled = (proto * (sc / pnorm).astype(np.float32)[:, None])
    pt = np.ascontiguousarray(
        p_scaled.reshape(NCT, 128, NK, 128).transpose(3, 0, 2, 1)
    ).astype(np.float16).reshape(128, NCT * NK * 128)
    xnorm = np.linalg.norm(inputs.astype(np.float64), axis=1)
    xinv_full = (10.0 / xnorm).astype(np.float32)

    in_maps = []
    for c in range(NCORES):
        xs = inputs[c * BS:(c + 1) * BS]
        xt = np.ascontiguousarray(
            xs.reshape(NB, 128, NK, 128).transpose(3, 0, 2, 1)
        ).astype(np.float16).reshape(128, NB * NK * 128)
        xinv = xinv_full[c * BS:(c + 1) * BS].reshape(NB, 128).T.copy()
        in_maps.append({"xt": xt, "pt": pt, "xinv": xinv})
    res = run_bass_kernel_spmd(nc, in_maps, core_ids=list(range(NCORES)),
                               trace=_trace, **_tr_kw)
    out = np.concatenate([res.results[c]["out"] for c in range(NCORES)],
                         axis=0).astype(np.float32)
    if _trace:
        kernel.last_results = res
    return out


# revision 39
# speedup vs baseline: 1.2333x; 1.0046x over previous
"""TRN2 Bass kernel for nn_CosClassifier: sim = 10*scalar * cos_sim(inputs, proto).

Data-parallel over 8 NeuronCores: each core computes a (2048, 4096) slab of
the (16384, 4096) similarity matrix as one fp16 GEMM. The device does all the
O(B*C*D) work -- the 17 GFLOP matmul, the 8.4M-element scaled PSUM drains and
every byte of output DMA; the host does O(input-size) preparation (sharding,
inverse norms, operand layout/dtype prep, fp16->f32 upcast of the result).

v6 shape, driven by trace analysis of on-device-prep versions (which plateaued
at ~97us):
  1. Output is fp16 in DRAM (16MB/core, the roofline stream) and upcast on the
     host; result quantization adds ~5e-4 relative error, inside the gate.
  2. Operands are uploaded GEMM-ready: x and proto pre-transposed to the
     [contract-dim partition, tile, k, free] stationary/moving layouts the PE
     wants, fp16, proto rows pre-scaled by scalar/||p||. This removes the PE
     transposes (8.4us), operand casts and proto scaling that previously kept
     ACT/DVE/PE mutually blocking, and halves input wire traffic (3MB/core).
  3. 10/||x_b|| is applied on-device in the PSUM->SBUF drains (per-partition
     activation scale, same cost as a plain copy). b-tiles are drained in
     PAIRS into one [128, 2048] staging tile -- ACT takes one half, DVE the
     other (different PSUM banks, legal in parallel) -- and ONE 512KB DMA
     ships the pair: half the ring issues/semaphores of per-tile DMAs.
  4. A burst of matmuls on a memset scratch tile warms the PE HAM clock gate
     (1.2 -> 2.4 GHz needs ~3.4us of sustained activity) while the first
     operand chunks stream in, so the real GEMM starts warm at ~11us.
"""
import sys

sys.path.insert(0, "/opt/trn_rl_repo")

import numpy as np

B, C, D = 16384, 4096, 256
NCORES = 8
BS = B // NCORES          # 2048 rows per core
NB = BS // 128            # 16 b-tiles per core
NCT = C // 128            # 32 c-tiles (proto rows)
NK = D // 128             # 2 k-tiles
NPH = 4                   # output column phases (1024 wide each)

_compiled = None


def _build():
    import concourse.bacc as bacc
    import concourse.mybir as mybir
    import concourse.tile as tile

    f32 = mybir.dt.float32
    f16 = mybir.dt.float16
    Act = mybir.ActivationFunctionType

    nc = bacc.Bacc("TRN2", target_bir_lowering=False, debug=False,
                   num_devices=NCORES)

    # xt[p, i, k, f] = x[i*128+f, k*128+p] : stationary layout, fp16
    xt_d = nc.dram_tensor("xt", [128, NB * NK * 128], f16,
                          kind="ExternalInput").ap()
    # pt[p, j, k, f] = (scalar/||p_c||)*proto[j*128+f, k*128+p] : moving
    pt_d = nc.dram_tensor("pt", [128, NCT * NK * 128], f16,
                          kind="ExternalInput").ap()
    xi_d = nc.dram_tensor("xinv", [128, NB], f32, kind="ExternalInput").ap()
    out_d = nc.dram_tensor("out", [BS, C], f16, kind="ExternalOutput").ap()

    with tile.TileContext(nc) as tc:
        with tc.tile_pool(name="sbuf", bufs=1) as pool, \
             tc.tile_pool(name="outp", bufs=8) as outp, \
             tc.tile_pool(name="psum_w", bufs=1, space="PSUM") as psum_w, \
             tc.tile_pool(name="psum_m", bufs=3, space="PSUM") as psum_m:

            xinv = pool.tile([128, NB], f32, tag="xinv")
            nc.scalar.dma_start(xinv[:], xi_d[:, :])

            # PE HAM warm-up on a memset scratch tile: no DMA dependency, so
            # the clock gate lifts to 2.4GHz while operands stream in
            wscr = pool.tile([128, 128], f16, tag="wscr")
            nc.gpsimd.memset(wscr[:], 0.0)
            wp = psum_w.tile([128, 512], f32, tag="wp")
            for _ in range(32):
                nc.tensor.matmul(wp[:, 0:128], wscr[:], wscr[:],
                                 start=True, stop=True)

            # ONE TILE PER DMA CHUNK: dependency tracking is tile-granular,
            # so a shared tile would make the first matmul wait for the LAST
            # chunk. All loads ride the sync ring in first-use order (a
            # second ring just contends for the same HBM wire); the first
            # matmul needs only xt chunk 0 (b-tiles 0-3) and pt chunk 0a
            # (n-block 0), 256KB each, so the GEMM starts at ~9.5us.
            # xt chunk tile-starts (in b-tiles): 4 tiles per chunk; smaller
            # first chunks lose more to the ~0.7us flat ring-issue cost per
            # DMA (which delays the output stream) than they gain in arrival
            XS = [0, 4, 8, 12, 16]
            xtc = [pool.tile([128, (XS[c + 1] - XS[c]) * NK * 128], f16,
                             tag=f"xt{c}", name=f"xt{c}")
                   for c in range(len(XS) - 1)]
            ptc = [pool.tile([128, 1024 if h < 2 else 2048], f16,
                             tag=f"pt{h}", name=f"pt{h}")
                   for h in range(NPH + 1)]
            nc.sync.dma_start(xtc[0][:], xt_d[:, :XS[1] * 256])
            nc.sync.dma_start(ptc[0][:], pt_d[:, :1024])
            nc.sync.dma_start(ptc[1][:], pt_d[:, 1024:2048])
            for c in range(1, len(XS) - 1):
                nc.sync.dma_start(
                    xtc[c][:], xt_d[:, XS[c] * 256:XS[c + 1] * 256])
            nc.sync.dma_start(ptc[2][:], pt_d[:, 2048:4096])
            # ptc[3]/ptc[4] (phases 2-3, needed ~30us later) are emitted
            # mid-stream so they queue BEHIND the first output DMAs on the
            # ring instead of delaying them
            chunk_of = []
            local_of = []
            for c in range(len(XS) - 1):
                for i in range(XS[c], XS[c + 1]):
                    chunk_of.append(c)
                    local_of.append(i - XS[c])

            xtc_r = [t[:].rearrange("p (i two d) -> p i two d", two=NK, d=128)
                     for t in xtc]
            ptc_r = [t[:].rearrange("p (j two d) -> p j two d", two=NK, d=128)
                     for t in ptc]

            def moving(h, nn_):
                # phase 0 n-blocks live in two half tiles; later phases in
                # one tile per phase
                if h == 0:
                    return ptc_r[nn_][:, 0:4, :, :]
                return ptc_r[h + 1][:, 4 * nn_:4 * nn_ + 4, :, :]

            # ---- main matmul + scaled fp16 drain ----
            # phase h covers n-blocks {2h, 2h+1}; per b-tile i: 4 fp16 MMs
            # (k-outer) into a 2-bank PSUM tile, one 1024-wide drain applying
            # 10/||x_b|| and casting to fp16. b-tiles are paired: ACT drains
            # one half, DVE the other (parallel, different banks), one 512KB
            # DMA per pair.
            def mm(h, i, oq2):
                ps = psum_m.tile([128, 1024], f32, tag="mm")
                for nn_ in range(2):
                    mv = moving(h, nn_)
                    for k in range(NK):
                        nc.tensor.matmul(
                            ps[:, nn_ * 512:(nn_ + 1) * 512],
                            xtc_r[chunk_of[i]][:, local_of[i], k, :],
                            mv[:, :, k, :],
                            start=(k == 0), stop=(k == NK - 1))
                inv = xinv[:, i:i + 1]
                half = oq2[:, (i % 2) * 1024:(i % 2 + 1) * 1024]
                if i % 2 == 0:
                    nc.scalar.activation(half, ps[:], Act.Copy, scale=inv)
                else:
                    nc.vector.tensor_scalar_mul(half, ps[:], inv)

            for h in range(NPH):
                for pr in range(NB // 2):
                    oq2 = outp.tile([128, 2048], f16, tag="oq")
                    for half in range(2):
                        mm(h, 2 * pr + half, oq2)
                    nc.sync.dma_start(
                        out_d[2 * pr * 128:(2 * pr + 2) * 128,
                              h * 1024:(h + 1) * 1024].rearrange(
                                  "(n p) c -> p n c", p=128),
                        oq2[:].rearrange("p (n c) -> p n c", c=1024))
                    if h == 0 and pr == 3:
                        nc.sync.dma_start(ptc[3][:], pt_d[:, 4096:6144])
                    if h == 0 and pr == 5:
                        nc.sync.dma_start(ptc[4][:], pt_d[:, 6144:8192])

    nc.compile()
    return nc


def _get_compiled():
    global _compiled
    if _compiled is None:
        _compiled = _build()
    return _compiled


def kernel(inputs, proto, scalar, _trace=False, **_tr_kw):
    from concourse.bass_utils import run_bass_kernel_spmd

    nc = _get_compiled()
    inputs = np.ascontiguousarray(inputs, dtype=np.float32)
    proto = np.ascontiguousarray(proto, dtype=np.float32)
    sc = float(np.asarray(scalar).reshape(-1)[0])

    # O(input-size) prep: inverse norms, proto pre-scale, and the PE
    # stationary/moving fp16 layouts ([p, tile, k, f], contract dim on the
    # partition axis)
    pnorm = np.linalg.norm(proto.astype(np.float64), axis=1)
    p_scaled = (proto * (sc / pnorm).astype(np.float32)[:, None])
    pt = np.ascontiguousarray(
        p_scaled.reshape(NCT, 128, NK, 128).transpose(3, 0, 2, 1)
    ).astype(np.float16).reshape(128, NCT * NK * 128)
    xnorm = np.linalg.norm(inputs.astype(np.float64), axis=1)
    xinv_full = (10.0 / xnorm).astype(np.float32)

    in_maps = []
    for c in range(NCORES):
        xs = inputs[c * BS:(c + 1) * BS]
        xt = np.ascontiguousarray(
            xs.reshape(NB, 128, NK, 128).transpose(3, 0, 2, 1)
        ).astype(np.float16).reshape(128, NB * NK * 128)
        xinv = xinv_full[c * BS:(c + 1) * BS].reshape(NB, 128).T.copy()
        in_maps.append({"xt": xt, "pt": pt, "xinv": xinv})
    res = run_bass_kernel_spmd(nc, in_maps, core_ids=list(range(NCORES)),
                               trace=_trace, **_tr_kw)
    out = np.concatenate([res.results[c]["out"] for c in range(NCORES)],
                         axis=0).astype(np.float32)
    if _trace:
        kernel.last_results = res
    return out
